# revision 19
# baseline (speedup 1.0000x reference)
"""ArbSR (moe_routing) Trainium2 kernel, 8-core SPMD.

Structure exploited: with scale=4, the scale-embedding MLP input is periodic
with period 4 in both HR axes, so routing r, offsets off, and the expert-mix
matrices take only 16 distinct values (one per (y%4, x%4) class).  The
offset grid_sample then becomes, per class, a 2x2-tap bilinear filter of the
encoder feature map f at a constant integer shift, and the whole
  fea0 -> expert mixing -> (+fea0) -> 3x3 tail conv
chain collapses to
  pred[:, 4*yl+b, 4*xl+a] = tail_b + sum_delta E[(b,a)][delta] @ f[:, yl+dy, xl+dx]
with host-precomputed [3,64] matrices E (a 3x3 delta neighborhood in
practice).  Tail-conv zero padding at the image border is handled with
per-edge correction streams; the right-edge correction rides the main
matmuls as extra stationary columns (M 64:112) and is applied from PSUM at
output column W-1; top/bottom corrections (with corner add-backs) and the
left edge are separate small matmuls whose weights are zeroed on cores
that don't own the edge.

Per core (64 HR rows): encoder conv as one K=56 block-diagonal matmul per
512-column chunk from a host-built doubled im2col (computes f and its
one-LR-row-shifted copy in a single pass); 6 K=128-packed main matmul
streams per bank (pred PSUM double-buffered across banks); the merged
pred tile [48, 2048] ships straight to DRAM per bank, and the host does
the nearest-neighbour query lookup (it already computes the query
indices to route them).  Keeping the query gather off-device removes the
PE transposes, the D-scratch DMA round trip, and the indirect-gather
chain that otherwise serialize the kernel tail.

Notes from measurement on the axon-tunneled cores: HWDGE dma_start
occupies its issuing engine ~0.8us and input DMA receipts take
~2.5-3us after issue, so the input DMAs are issued in parallel across
all five engines right at kernel start; the NEFF's fixed semaphore-reset
epilogue (~7us, PE-bound) is outside kernel control.
"""

import numpy as np
import ml_dtypes

BF16 = ml_dtypes.bfloat16


def _ensure_path():
    import sys
    for p in ('/opt/trn_rl_repo',):
        if p not in sys.path:
            sys.path.append(p)


H = W = 128
S = 4
HH = WH = H * S          # 512
C = 64
NCORES = 8
YLC = H // NCORES        # 16 LR rows per core
HRPC = HH // NCORES      # 64 HR rows per core
NCLS = 16                # (b, a) classes
MROWS = NCLS * 3         # 48 stacked pred rows
RIG0 = 64                # right-edge corr block base (32-aligned for DVE)
MW = RIG0 + MROWS        # main lhsT cols: 0:48 pred, 64:112 right-edge corr


def _sigmoid(x):
    return 1.0 / (1.0 + np.exp(-x))


def _class_constants(d):
    w1 = np.asarray(d['body_w1'], np.float64)
    b1 = np.asarray(d['body_b1'], np.float64)
    w2 = np.asarray(d['body_w2'], np.float64)
    b2 = np.asarray(d['body_b2'], np.float64)
    rw = np.asarray(d['routing_w'], np.float64)
    rb = np.asarray(d['routing_b'], np.float64)
    ow = np.asarray(d['offset_w'], np.float64)
    ob = np.asarray(d['offset_b'], np.float64)
    wc = np.asarray(d['weight_compress'], np.float64)
    we = np.asarray(d['weight_expand'], np.float64)

    fs = float(S)
    coor = np.array([(i + 0.5) / fs - np.floor((i + 0.5) / fs + 0.001) - 0.5
                     for i in range(S)])
    cls = {}
    for b in range(S):
        for a in range(S):
            inp4 = np.array([1.0 / fs, 1.0 / fs, coor[b], coor[a]])
            emb = np.maximum(w1 @ inp4 + b1, 0.0)
            emb = np.maximum(w2 @ emb + b2, 0.0)
            off = ow @ emb + ob
            r = _sigmoid(rw @ emb + rb)
            A = np.einsum('e,eck->ck', r, we) @ np.einsum('e,ekc->kc', r, wc)
            B = A + np.eye(C)
            cx = (a + 0.5) / fs - 0.5 + off[0]
            cy = (b + 0.5) / fs - 0.5 + off[1]
            ix, iy = int(np.floor(cx)), int(np.floor(cy))
            fx, fy = cx - ix, cy - iy
            wbl = {(0, 0): (1 - fy) * (1 - fx), (0, 1): (1 - fy) * fx,
                   (1, 0): fy * (1 - fx), (1, 1): fy * fx}
            cls[(b, a)] = dict(B=B, ix=ix, iy=iy, wbl=wbl)
    return cls


def _build_E(tail_w, cls, only_ty=None, only_tx=None):
    """E[(b,a)][(dy,dx)] = [3, C] so that pred contribution is E @ f(shift)."""
    Es = {}
    for b in range(S):
        for a in range(S):
            acc = {}
            for ty in range(3):
                if only_ty is not None and ty not in only_ty:
                    continue
                for tx in range(3):
                    if only_tx is not None and tx not in only_tx:
                        continue
                    bp = (b + ty - 1) % S
                    oy = (b + ty - 1 - bp) // S
                    ap_ = (a + tx - 1) % S
                    ox = (a + tx - 1 - ap_) // S
                    c2 = cls[(bp, ap_)]
                    TB = tail_w[:, :, ty, tx] @ c2['B']
                    for (uy, ux), wgt in c2['wbl'].items():
                        if wgt == 0.0:
                            continue
                        key = (oy + c2['iy'] + uy, ox + c2['ix'] + ux)
                        acc[key] = acc.get(key, np.zeros((3, C))) + TB * wgt
            Es[(b, a)] = acc
    return Es


def _stack_E(Es, deltas, classes=None, sign=1.0):
    """Per-delta [MROWS, C] matrices, rows ordered (4b+a)*3 + c."""
    G = {dl: np.zeros((MROWS, C)) for dl in deltas}
    for (b, a), acc in Es.items():
        if classes is not None and (b, a) not in classes:
            continue
        m0 = (4 * b + a) * 3
        for dl, M in acc.items():
            if dl not in G:
                continue
            G[dl][m0:m0 + 3, :] += sign * M
    return G


def _pair_streams(deltas):
    """Pair (dy,dx) with (dy+1,dx); unpaired run as K=64 streams."""
    deltas = sorted(deltas)
    dset, used, streams = set(deltas), set(), []
    for dl in deltas:
        if dl in used:
            continue
        hi = (dl[0] + 1, dl[1])
        if hi in dset and hi not in used:
            streams.append((dl, True))
            used.update((dl, hi))
        else:
            streams.append((dl, False))
            used.add(dl)
    return streams


def _make_main_streams(deltas):
    """Main-stream specs minimizing PE passes: y-pairs via the doubled
    f2 stack, then x-pairs among the leftovers via the column-shifted g
    stack, then true singles.  spec = (kind, base_delta) with kind
    'fy' (K=128, limbs (dy,dx)+(dy+1,dx) from f3),
    'gx' (K=128, limbs (dy,dx)+(dy,dx+1) from g3),
    'f1' (K=64 single from f3).  'gx' specs go last so the g copies
    have time to land."""
    dset, used = set(deltas), set()
    fy, f1 = [], []
    for dl in sorted(deltas):
        if dl in used:
            continue
        hi = (dl[0] + 1, dl[1])
        if hi in dset and hi not in used:
            fy.append(('fy', dl))
            used.update((dl, hi))
    for dl in sorted(deltas):
        if dl not in used:
            f1.append(('f1', dl))
            used.add(dl)
    return fy + f1


def _stream_tensors(G, streams, mw=MROWS, G2=None):
    """lhsT arrays [K, mw] per stream (K=128 paired, 64 single).

    With G2, cols 64:112 carry the second stack (right-edge corr)."""
    out = []
    for dl, paired in streams:
        def block(d):
            M = np.zeros((mw, C))
            M[0:MROWS, :] = G[d]
            if G2 is not None and d in G2:
                M[RIG0:RIG0 + MROWS, :] += G2[d]
            return M
        if paired:
            hi = (dl[0] + 1, dl[1])
            lhsT = np.zeros((128, mw), np.float32)
            lhsT[0:C, :] = block(dl).T
            lhsT[C:2 * C, :] = block(hi).T
        else:
            lhsT = np.ascontiguousarray(block(dl).T, dtype=np.float32)
        out.append(lhsT)
    return out


def _plan_and_host_data(d):
    """Everything the host precomputes: stream plans, per-core inputs,
    query indices."""
    cls = _class_constants(d)
    tail_w = np.asarray(d['tail_w'], np.float64)
    tail_b = np.asarray(d['tail_b'], np.float64)

    E_main = _build_E(tail_w, cls)
    deltas = sorted({k for acc in E_main.values() for k in acc})
    dys = [dl[0] for dl in deltas]
    dxs = [dl[1] for dl in deltas]
    dy_min, dy_max = min(dys), max(dys)
    dx_min, dx_max = min(dxs), max(dxs)
    NRF = 17 + dy_max - dy_min       # f rows per core (last row upper-only)
    NCF = W + dx_max - dx_min        # f cols
    NF = NRF * NCF
    assert NRF <= 40 and NCF <= 192, (NRF, NCF)

    main_streams = _make_main_streams(deltas)
    G_main = _stack_E(E_main, deltas)
    # right-edge correction folded into main lhsT cols 64:112 (sign -1),
    # read from the matmul output at column W-1 of each row
    rig_cls = [(b, 3) for b in range(S)]
    E_rig = _build_E(tail_w, cls, only_tx=(2,))
    G_rig = _stack_E(E_rig, deltas, classes=rig_cls, sign=-1.0)

    def main_block(d):
        M = np.zeros((MW, C))
        M[0:MROWS, :] = G_main[d]
        if d in G_rig:
            M[RIG0:RIG0 + MROWS, :] += G_rig[d]
        return M

    main_T = []
    for kind, dl in main_streams:
        if kind == 'f1':
            lhsT = np.ascontiguousarray(main_block(dl).T, dtype=np.float32)
        else:
            hi = ((dl[0] + 1, dl[1]) if kind == 'fy'
                  else (dl[0], dl[1] + 1))
            lhsT = np.zeros((128, MW), np.float32)
            lhsT[0:C, :] = main_block(dl).T
            lhsT[C:2 * C, :] = main_block(hi).T
        main_T.append(lhsT)

    # remaining edge corrections as separate streams
    def corr(only_ty, only_tx, classes, sign):
        E = _build_E(tail_w, cls, only_ty=only_ty, only_tx=only_tx)
        dls = sorted({k for (ba, acc) in E.items() if ba in classes
                      for k, M in acc.items()})
        if not dls:
            return [], []
        st = _pair_streams(dls)
        G = _stack_E(E, dls, classes=classes, sign=sign)
        return st, _stream_tensors(G, st)

    top_cls = [(0, a) for a in range(S)]
    bot_cls = [(3, a) for a in range(S)]
    lef_cls = [(b, 0) for b in range(S)]
    c_lef = corr(None, (0,), lef_cls, -1.0)

    # top/bottom edge correction M-stacked with the corner add-backs:
    # lhsT cols 0:48 = edge correction; cols 64:96 / 96:128 hold the two
    # corner add-backs at 32-aligned windows (row j of window <-> pred row
    # j + base), so corner merges are 32-partition-aligned DVE adds.
    def corr_merged(only_ty, edge_cls, cornerL, cornerR, cbase):
        E_edge = _build_E(tail_w, cls, only_ty=only_ty)
        E_cl = _build_E(tail_w, cls, only_ty=only_ty, only_tx=(0,))
        E_cr = _build_E(tail_w, cls, only_ty=only_ty, only_tx=(2,))
        dls = sorted({k for ba in edge_cls for k in E_edge[ba]})
        st = _pair_streams(dls)
        G = {dl: np.zeros((128, C)) for dl in dls}
        for ba in edge_cls:
            m0 = (4 * ba[0] + ba[1]) * 3
            for dl, M in E_edge[ba].items():
                G[dl][m0:m0 + 3, :] -= M
        mL = (4 * cornerL[0] + cornerL[1]) * 3 - cbase
        for dl, M in E_cl[cornerL].items():
            G[dl][64 + mL:64 + mL + 3, :] += M
        mR = (4 * cornerR[0] + cornerR[1]) * 3 - cbase
        for dl, M in E_cr[cornerR].items():
            G[dl][96 + mR:96 + mR + 3, :] += M
        out = []
        for dl, paired in st:
            if paired:
                hi = (dl[0] + 1, dl[1])
                lhsT = np.zeros((128, 128), np.float32)
                lhsT[0:C, :] = G[dl].T
                lhsT[C:2 * C, :] = G.get(hi, np.zeros((128, C))).T
            else:
                lhsT = np.ascontiguousarray(G[dl].T, dtype=np.float32)
            out.append(lhsT)
        return st, out

    # top window base 0 (corners in pred rows 0-31), bottom base 32
    c_top = corr_merged((0,), top_cls, (0, 0), (0, 3), 0)
    c_bot = corr_merged((2,), bot_cls, (3, 0), (3, 3), 32)

    zeros_like_T = lambda Ts: [np.zeros_like(t) for t in Ts]

    # encoder weights, block-diagonal over the two row-shifted halves:
    # K = 2*28 rows = (9 taps x 3 ch + bias row) x 2, M = 128 = f | f(y+1)
    enc_w = np.asarray(d['enc_w'], np.float64)
    enc_b = np.asarray(d['enc_b'], np.float64)
    encw = np.zeros((28, C), np.float32)
    for ty in range(3):
        for tx in range(3):
            for ch in range(3):
                encw[(ty * 3 + tx) * 3 + ch, :] = enc_w[:, ch, ty, tx]
    encw[27, :] = enc_b
    encw56 = np.zeros((56, 128), np.float32)
    encw56[0:28, 0:C] = encw
    encw56[28:56, C:128] = encw

    # per-core doubled im2col [56, NF]: rows 28:56 = one LR row down
    inp = np.asarray(d['inp'], np.float64)[0]   # [3, H, W]
    PADX = 64
    ippad = np.pad(inp, ((0, 0), (PADX, PADX), (PADX, PADX)))
    ones = np.zeros((H + 2 * PADX, W + 2 * PADX))
    ones[PADX:PADX + H, PADX:PADX + W] = 1.0
    im2cols = []
    for core in range(NCORES):
        y0 = YLC * core + dy_min          # global LR row of f-tile row 0
        x0 = dx_min
        NR1 = NRF + 1
        im = np.zeros((28, NR1, NCF), np.float32)
        for ty in range(3):
            for tx in range(3):
                ys = PADX + y0 + ty - 1
                xs = PADX + x0 + tx - 1
                for ch in range(3):
                    im[(ty * 3 + tx) * 3 + ch] = \
                        ippad[ch, ys:ys + NR1, xs:xs + NCF]
        inside = ones[PADX + y0:PADX + y0 + NR1, PADX + x0:PADX + x0 + NCF]
        im[27] = inside
        # f must be exactly zero at out-of-image positions (grid-sample
        # zero padding), so kill whole columns there, not just oob taps
        im *= inside[None].astype(np.float32)
        imf = im.reshape(28, NR1 * NCF)
        im56 = np.zeros((56, NF), np.float32)
        im56[0:28] = imf[:, 0:NF]
        im56[28:56] = imf[:, NCF:NF + NCF]
        im2cols.append(im56.astype(BF16))

    # query indices (f32 math matches reference rounding); the value
    # lookup happens on host after the kernel ships pred
    coord = np.asarray(d['coord'], np.float32)[0]
    cell = np.asarray(d['cell'], np.float32)[0]
    cq = np.clip(coord - cell * np.float32(0.5) + np.float32(1e-6),
                 np.float32(-1 + 1e-6), np.float32(1 - 1e-6))
    xi = np.clip(np.round((cq[:, 1] + 1) * np.float32(0.5) * (WH - 1)
                          ).astype(np.int64), 0, WH - 1)
    yi = np.clip(np.round((cq[:, 0] + 1) * np.float32(0.5) * (HH - 1)
                          ).astype(np.int64), 0, HH - 1)

    bias48 = np.zeros((1, MROWS), np.float32)
    for b in range(S):
        for a in range(S):
            bias48[0, (4 * b + a) * 3:(4 * b + a) * 3 + 3] = tail_b

    plan = dict(
        dy_min=dy_min, dx_min=dx_min, NRF=NRF, NCF=NCF, NF=NF,
        main_streams=main_streams,
        corr_specs=dict(top=c_top[0], bot=c_bot[0], lef=c_lef[0]),
    )

    per_core_corr = []
    for core in range(NCORES):
        cc = dict(lef=c_lef[1])
        cc['top'] = c_top[1] if core == 0 else zeros_like_T(c_top[1])
        cc['bot'] = c_bot[1] if core == NCORES - 1 else zeros_like_T(c_bot[1])
        per_core_corr.append(cc)

    # ---- pack every small constant into one [128, CW] blob ----
    # segment name -> (p0, c0, K, Mw)
    segs = {}
    state = dict(col=0)

    def alloc(name, K, Mw):
        c0 = state['col']
        segs[name] = (0, c0, K, Mw)
        state['col'] += Mw
        return segs[name]

    alloc('encw', 56, 128)
    alloc('bias48', MROWS, 1)
    CW1 = state['col']               # chunk 1: encw + bias
    for s, t in enumerate(main_T):
        alloc(f'Em{s}', t.shape[0], MW)
    CW2 = state['col']               # chunk 2: Em streams
    for name, streams, mw in [('top', c_top[0], 128), ('bot', c_bot[0], 128),
                              ('lef', c_lef[0], MROWS)]:
        for s, (dl, paired) in enumerate(streams):
            alloc(f'Ec_{name}{s}', 128 if paired else C, mw)
    CW = state['col']
    plan['segs'] = segs
    plan['CW'] = CW
    plan['CW1'] = CW1
    plan['CW2'] = CW2

    consts_cores = []
    for core in range(NCORES):
        blob = np.zeros((128, CW), np.float32)

        def put(name, arr):
            p0, c0, K, Mw = segs[name]
            blob[p0:p0 + arr.shape[0], c0:c0 + arr.shape[1]] = arr

        put('encw', encw56)
        put('bias48', bias48.reshape(MROWS, 1))
        for s, t in enumerate(main_T):
            put(f'Em{s}', t)
        for name, Ts in per_core_corr[core].items():
            for s, t in enumerate(Ts):
                put(f'Ec_{name}{s}', t)
        consts_cores.append(blob.astype(BF16))

    host = dict(consts=consts_cores, im2cols=im2cols, xi=xi, yi=yi,
                Q=coord.shape[0])
    return plan, host


def _build_graph(plan, host, debug_outputs=False):
    _ensure_path()
    import concourse.bass as bass
    import concourse.bacc as bacc
    import concourse.mybir as mybir
    import concourse.tile as tile

    f32 = mybir.dt.float32
    bf16 = mybir.dt.bfloat16

    NRF, NCF, NF = plan['NRF'], plan['NCF'], plan['NF']
    dy_min, dx_min = plan['dy_min'], plan['dx_min']
    main_streams = plan['main_streams']
    corr_specs = plan['corr_specs']
    segs, CW = plan['segs'], plan['CW']

    nc = bacc.Bacc(None, target_bir_lowering=False, debug=False,
                   num_devices=NCORES)

    im2col_d = nc.dram_tensor('im2col', [56, NF], bf16, kind='ExternalInput')
    consts_d = nc.dram_tensor('consts', [128, CW], bf16,
                              kind='ExternalInput')
    out_d = nc.dram_tensor('out', [MROWS, YLC * W], bf16,
                           kind='ExternalOutput')
    if debug_outputs:
        dbg_f2 = nc.dram_tensor('dbg_f2', [128, NF], f32, kind='ExternalOutput')

    with tile.TileContext(nc) as tc:
        with (
            tc.tile_pool(name='sb', bufs=1) as sb,
            tc.tile_pool(name='sbsmall', bufs=1) as sbs,
            tc.tile_pool(name='pshare', bufs=4, space='PSUM') as pshare,
            tc.tile_pool(name='ppred', bufs=3, space='PSUM') as ppred,
            tc.tile_pool(name='pcorr', bufs=1, space='PSUM') as pcorr,
        ):
            consts_t = sb.tile([128, CW], bf16)
            im2col = sb.tile([56, NF], bf16)

            # input DMAs fan out across the three DMA-capable engines
            # (sync/scalar/gpsimd) in criticality order: the HWDGE
            # descriptor-gen (~0.8us each) serializes per engine and
            # receipts land ~2.5-3us after issue.
            CW1, CW2 = plan['CW1'], plan['CW2']
            CH = 512
            nchunks = (NF + CH - 1) // CH
            nc.sync.dma_start(im2col[:, 0:CH], im2col_d[:, 0:CH])
            nc.scalar.dma_start(consts_t[:, 0:CW1], consts_d[:, 0:CW1])
            nc.gpsimd.dma_start(consts_t[:, CW1:CW2], consts_d[:, CW1:CW2])
            nc.sync.dma_start(im2col[:, CH:2 * CH], im2col_d[:, CH:2 * CH])
            nc.scalar.dma_start(im2col[:, 2 * CH:3 * CH],
                                im2col_d[:, 2 * CH:3 * CH])
            nc.gpsimd.dma_start(im2col[:, 3 * CH:4 * CH],
                                im2col_d[:, 3 * CH:4 * CH])
            nc.sync.dma_start(im2col[:, 4 * CH:NF], im2col_d[:, 4 * CH:NF])
            nc.scalar.dma_start(consts_t[:, CW2:CW], consts_d[:, CW2:CW])

            # warm-up matmuls fill the input-DMA wait: the PE clock ramps
            # with activity (~0.8 GHz cold -> ~1.2 GHz warm), so burning
            # the dead time on dummy matmuls pulls the fast clock earlier
            warm = sbs.tile([128, 512], bf16)
            nc.vector.memset(warm[:], 0)
            for _ in range(6):
                pw = pshare.tile([128, 512], f32, tag='pshare')
                nc.tensor.matmul(pw[:], warm[:, 0:128], warm[:],
                                 start=True, stop=True,
                                 skip_group_check=True)

            def cseg(name):
                p0, c0, K, Mw = segs[name]
                return consts_t[p0:p0 + K, c0:c0 + Mw]

            encw_t = cseg('encw')
            bias48_t = cseg('bias48')
            biasf_t = sbs.tile([MROWS, 1], f32)
            mainT_t = [cseg(f'Em{s}') for s in range(len(main_streams))]
            corrT_t = {name: [cseg(f'Ec_{name}{s}')
                              for s in range(len(streams))]
                       for name, streams in corr_specs.items()}

            # encoder conv: f2 = [f ; f shifted one LR row] in one
            # block-diagonal K=56 matmul per chunk.  The PSUM->SBUF cast
            # splits across vector and scalar: big SBUF writes slow
            # concurrent main-matmul SBUF reads ~1.5x, so compressing
            # the cast chain shortens the contention window.
            f2 = sb.tile([128, NF], bf16)

            def enc_chunk(ci):
                n0, n1 = ci * CH, min(NF, (ci + 1) * CH)
                nh = (n1 - n0) // 2
                pe = pshare.tile([128, CH], f32, tag='pshare')
                nc.tensor.matmul(pe[:, :n1 - n0],
                                 encw_t,
                                 im2col[:, n0:n1],
                                 start=True, stop=True,
                                 skip_group_check=True)
                nc.vector.tensor_copy(f2[:, n0:n0 + nh], pe[:, :nh])
                nc.scalar.activation(f2[:, n0 + nh:n1], pe[:, nh:n1 - n0],
                                     mybir.ActivationFunctionType.Copy)

            f3 = f2[:].rearrange('p (r c) -> p r c', c=NCF)

            def main_mms(nb, pred_ps):
                for s, (kind, dl) in enumerate(main_streams):
                    K = C if kind == 'f1' else 128
                    r0 = 4 * nb + dl[0] - dy_min
                    c0 = dl[1] - dx_min
                    nc.tensor.matmul(
                        pred_ps[:],
                        mainT_t[s],
                        f3[0:K, r0:r0 + 4, c0:c0 + W],
                        start=(s == 0), stop=(s == len(main_streams) - 1),
                        skip_group_check=True)

            # chunks 0-2 cover bank 0's rows: its mains slot between the
            # encoder chunks so late im2col receipts hide behind them
            for ci in range(3):
                enc_chunk(ci)
            pred_ps0 = ppred.tile([MW, 512], f32, tag='ppred')
            main_mms(0, pred_ps0)
            for ci in range(3, nchunks):
                enc_chunk(ci)

            # corrections first: they only need f2, and every bank's
            # merge depends on them; top/bot (with corner windows) and
            # lef share one PSUM bank
            corrall = pcorr.tile([128, 512], f32)
            corr2_ps = corrall[:, 0:256]
            corr_ps = corrall[0:MROWS, 256:272]

            def corr_mms(name, col0, row_sel, col_sel, nfree, ps):
                streams = corr_specs[name]
                if not streams:
                    return False
                for s, (dl, paired) in enumerate(streams):
                    K = 128 if paired else C
                    r0 = row_sel + dl[0] - dy_min
                    c0 = col_sel + dl[1] - dx_min
                    if nfree == 128:     # one row, all cols
                        rhs = f3[0:K, r0:r0 + 1, c0:c0 + W]
                    else:                # all rows, one col
                        rhs = f3[0:K, r0:r0 + YLC, c0:c0 + 1]
                    nc.tensor.matmul(
                        ps[:, col0:col0 + nfree],
                        corrT_t[name][s],
                        rhs,
                        start=(s == 0), stop=(s == len(streams) - 1),
                        skip_group_check=True)
                return True

            has = dict()
            has['top'] = corr_mms('top', 0, 0, 0, 128, corr2_ps)
            has['bot'] = corr_mms('bot', 128, YLC - 1, 0, 128, corr2_ps)
            has['lef'] = corr_mms('lef', 0, 0, 0, 16, corr_ps)

            # emitted here (not at consts load) so the wait on the consts
            # receipt never head-of-line-blocks the encoder copies
            nc.vector.tensor_copy(biasf_t[:], bias48_t)

            # fused per-bank pipeline: main matmuls -> copy+merge -> out
            # store; pred PSUM is double-buffered so the next bank's
            # matmuls overlap this bank's merge reads.  Banks run in
            # order [0,1,3,2] so the bottom-edge bank (3, with its extra
            # corner adds) is never the last one on the critical tail.
            pred_sb = sb.tile([MROWS, YLC * W], bf16)
            p4 = pred_sb[:].rearrange('p (r c) -> p r c', c=W)

            for nb in (0, 1, 3, 2):
                if nb == 0:
                    pred_ps = pred_ps0
                else:
                    pred_ps = ppred.tile([MW, 512], f32, tag='ppred')
                    main_mms(nb, pred_ps)
                rig4 = pred_ps[RIG0:RIG0 + MROWS, :].rearrange(
                    'p (r c) -> p r c', c=W)
                # both half-merges issue first (scalar || vector), then
                # the edge adds, so the add chain never waits on a merge
                r0a, r0b = 4 * nb, 4 * nb + 2
                nc.scalar.activation(
                    pred_sb[:, r0a * W:(r0a + 2) * W],
                    pred_ps[0:MROWS, 0:256],
                    mybir.ActivationFunctionType.Identity,
                    bias=biasf_t[:])
                nc.vector.tensor_scalar_add(
                    pred_sb[:, r0b * W:(r0b + 2) * W],
                    pred_ps[0:MROWS, 256:512],
                    biasf_t[:])
                for half in range(2):
                    r0, r1 = 4 * nb + 2 * half, 4 * nb + 2 * half + 2
                    if has['lef']:
                        nc.vector.tensor_add(
                            p4[:, r0:r1, 0:1],
                            p4[:, r0:r1, 0:1],
                            corr_ps[:, r0:r1]
                            .rearrange('p (r c) -> p r c', c=1))
                    # right-edge correction rides the main matmuls (M 64:112)
                    nc.vector.tensor_add(
                        p4[:, r0:r1, W - 1:W],
                        p4[:, r0:r1, W - 1:W],
                        rig4[:, 2 * half:2 * half + 2, W - 1:W])
                    if nb == 0 and half == 0 and has['top']:
                        nc.vector.tensor_add(pred_sb[:, 0:W],
                                             pred_sb[:, 0:W],
                                             corr2_ps[0:MROWS, 0:W])
                        nc.vector.tensor_add(
                            p4[0:32, 0:1, 0:1], p4[0:32, 0:1, 0:1],
                            corr2_ps[64:96, 0:1]
                            .rearrange('p (r c) -> p r c', c=1))
                        nc.vector.tensor_add(
                            p4[0:32, 0:1, W - 1:W], p4[0:32, 0:1, W - 1:W],
                            corr2_ps[96:128, W - 1:W]
                            .rearrange('p (r c) -> p r c', c=1))
                    if nb == 3 and half == 1 and has['bot']:
                        nc.vector.tensor_add(
                            pred_sb[:, (YLC - 1) * W:YLC * W],
                            pred_sb[:, (YLC - 1) * W:YLC * W],
                            corr2_ps[0:MROWS, 128:128 + W])
                        nc.vector.tensor_add(
                            p4[32:48, YLC - 1:YLC, 0:1],
                            p4[32:48, YLC - 1:YLC, 0:1],
                            corr2_ps[64:80, 128:129]
                            .rearrange('p (r c) -> p r c', c=1))
                        nc.vector.tensor_add(
                            p4[32:48, YLC - 1:YLC, W - 1:W],
                            p4[32:48, YLC - 1:YLC, W - 1:W],
                            corr2_ps[96:112, 255:256]
                            .rearrange('p (r c) -> p r c', c=1))
                    # out stores alternate queues so the last bank's two
                    # halves don't serialize on one descriptor generator
                    eng = nc.sync if half == 0 else nc.scalar
                    eng.dma_start(
                        out_d[:, r0 * W:r1 * W],
                        pred_sb[:, r0 * W:r1 * W])

            if debug_outputs:
                nc.sync.dma_start(dbg_f2[:], f2[:].bitcast(f32))

    nc.compile()
    return nc


def make_in_maps(host):
    in_maps = []
    for core in range(NCORES):
        m = {
            'im2col': host['im2cols'][core],
            'consts': host['consts'][core],
        }
        in_maps.append(m)
    return in_maps


def kernel(**inputs) -> np.ndarray:
    _ensure_path()
    from concourse.bass_utils import run_bass_kernel_spmd

    scale = inputs.get('scale', S)
    scale = int(np.asarray(scale)) if not isinstance(scale, int) else scale
    assert scale == S, f"kernel hardcodes scale={S}, got {scale}"

    plan, host = _plan_and_host_data(inputs)
    nc = _build_graph(plan, host)

    in_maps = make_in_maps(host)
    res = run_bass_kernel_spmd(nc, in_maps, core_ids=list(range(NCORES)))

    # assemble pred [3, HH, WH] from the per-core [48, YLC*W] tiles:
    # row (4b+a)*3+c, col yl*W+xl  ->  pred[c, HRPC*core + 4*yl + b, 4*xl + a]
    pred = np.empty((3, HH, WH), np.float32)
    for core in range(NCORES):
        t = np.asarray(res.results[core]['out']).astype(np.float32)
        t = t.reshape(S, S, 3, YLC, W)            # [b, a, c, yl, xl]
        pred[:, HRPC * core:HRPC * (core + 1), :] = (
            t.transpose(2, 3, 0, 4, 1).reshape(3, HRPC, WH))
    q = pred[:, host['yi'], host['xi']].T         # [Q, 3]
    return q[None]


# revision 21
# speedup vs baseline: 1.1273x; 1.1273x over previous
"""ArbSR (moe_routing) Trainium2 kernel, 8-core SPMD.

Structure exploited: with scale=4, the scale-embedding MLP input is periodic
with period 4 in both HR axes, so routing r, offsets off, and the expert-mix
matrices take only 16 distinct values (one per (y%4, x%4) class).  The
offset grid_sample then becomes, per class, a 2x2-tap bilinear filter of the
encoder feature map f at a constant integer shift, and the whole
  fea0 -> expert mixing -> (+fea0) -> 3x3 tail conv
chain collapses to
  pred[:, 4*yl+b, 4*xl+a] = tail_b + sum_delta E[(b,a)][delta] @ f[:, yl+dy, xl+dx]
with host-precomputed [3,64] matrices E (a 3x3 delta neighborhood in
practice).  Tail-conv zero padding at the image border is handled with
per-edge correction streams; the right-edge correction rides the main
matmuls as extra stationary columns (M 64:112) and is applied from PSUM at
output column W-1; top/bottom corrections (with corner add-backs) and the
left edge are separate small matmuls whose weights are zeroed on cores
that don't own the edge.

Per core (64 HR rows): encoder conv as one K=56 block-diagonal matmul per
512-column chunk from a host-built doubled im2col (computes f and its
one-LR-row-shifted copy in a single pass); 6 K=128-packed main matmul
streams per bank (pred PSUM double-buffered across banks); the merged
pred tile [48, 2048] ships straight to DRAM per bank, and the host does
the nearest-neighbour query lookup (it already computes the query
indices to route them).  Keeping the query gather off-device removes the
PE transposes, the D-scratch DMA round trip, and the indirect-gather
chain that otherwise serialize the kernel tail.

Notes from measurement on the axon-tunneled cores: HWDGE dma_start
occupies its issuing engine ~0.8us and input DMA receipts take
~2.5-3us after issue, so the input DMAs are issued in parallel across
all five engines right at kernel start; the NEFF's fixed semaphore-reset
epilogue (~7us, PE-bound) is outside kernel control.
"""

import numpy as np
import ml_dtypes

BF16 = ml_dtypes.bfloat16


def _ensure_path():
    import sys
    for p in ('/opt/trn_rl_repo',):
        if p not in sys.path:
            sys.path.append(p)


H = W = 128
S = 4
HH = WH = H * S          # 512
C = 64
NCORES = 8
YLC = H // NCORES        # 16 LR rows per core
HRPC = HH // NCORES      # 64 HR rows per core
NCLS = 16                # (b, a) classes
MROWS = NCLS * 3         # 48 stacked pred rows
RIG0 = 64                # right-edge corr block base (32-aligned for DVE)
MW = RIG0 + MROWS        # main lhsT cols: 0:48 pred, 64:112 right-edge corr


def _sigmoid(x):
    return 1.0 / (1.0 + np.exp(-x))


def _class_constants(d):
    w1 = np.asarray(d['body_w1'], np.float64)
    b1 = np.asarray(d['body_b1'], np.float64)
    w2 = np.asarray(d['body_w2'], np.float64)
    b2 = np.asarray(d['body_b2'], np.float64)
    rw = np.asarray(d['routing_w'], np.float64)
    rb = np.asarray(d['routing_b'], np.float64)
    ow = np.asarray(d['offset_w'], np.float64)
    ob = np.asarray(d['offset_b'], np.float64)
    wc = np.asarray(d['weight_compress'], np.float64)
    we = np.asarray(d['weight_expand'], np.float64)

    fs = float(S)
    coor = np.array([(i + 0.5) / fs - np.floor((i + 0.5) / fs + 0.001) - 0.5
                     for i in range(S)])
    cls = {}
    for b in range(S):
        for a in range(S):
            inp4 = np.array([1.0 / fs, 1.0 / fs, coor[b], coor[a]])
            emb = np.maximum(w1 @ inp4 + b1, 0.0)
            emb = np.maximum(w2 @ emb + b2, 0.0)
            off = ow @ emb + ob
            r = _sigmoid(rw @ emb + rb)
            A = np.einsum('e,eck->ck', r, we) @ np.einsum('e,ekc->kc', r, wc)
            B = A + np.eye(C)
            cx = (a + 0.5) / fs - 0.5 + off[0]
            cy = (b + 0.5) / fs - 0.5 + off[1]
            ix, iy = int(np.floor(cx)), int(np.floor(cy))
            fx, fy = cx - ix, cy - iy
            wbl = {(0, 0): (1 - fy) * (1 - fx), (0, 1): (1 - fy) * fx,
                   (1, 0): fy * (1 - fx), (1, 1): fy * fx}
            cls[(b, a)] = dict(B=B, ix=ix, iy=iy, wbl=wbl)
    return cls


def _build_E(tail_w, cls, only_ty=None, only_tx=None):
    """E[(b,a)][(dy,dx)] = [3, C] so that pred contribution is E @ f(shift)."""
    Es = {}
    for b in range(S):
        for a in range(S):
            acc = {}
            for ty in range(3):
                if only_ty is not None and ty not in only_ty:
                    continue
                for tx in range(3):
                    if only_tx is not None and tx not in only_tx:
                        continue
                    bp = (b + ty - 1) % S
                    oy = (b + ty - 1 - bp) // S
                    ap_ = (a + tx - 1) % S
                    ox = (a + tx - 1 - ap_) // S
                    c2 = cls[(bp, ap_)]
                    TB = tail_w[:, :, ty, tx] @ c2['B']
                    for (uy, ux), wgt in c2['wbl'].items():
                        if wgt == 0.0:
                            continue
                        key = (oy + c2['iy'] + uy, ox + c2['ix'] + ux)
                        acc[key] = acc.get(key, np.zeros((3, C))) + TB * wgt
            Es[(b, a)] = acc
    return Es


def _stack_E(Es, deltas, classes=None, sign=1.0):
    """Per-delta [MROWS, C] matrices, rows ordered (4b+a)*3 + c."""
    G = {dl: np.zeros((MROWS, C)) for dl in deltas}
    for (b, a), acc in Es.items():
        if classes is not None and (b, a) not in classes:
            continue
        m0 = (4 * b + a) * 3
        for dl, M in acc.items():
            if dl not in G:
                continue
            G[dl][m0:m0 + 3, :] += sign * M
    return G


def _pair_streams(deltas):
    """Pair (dy,dx) with (dy+1,dx); unpaired run as K=64 streams."""
    deltas = sorted(deltas)
    dset, used, streams = set(deltas), set(), []
    for dl in deltas:
        if dl in used:
            continue
        hi = (dl[0] + 1, dl[1])
        if hi in dset and hi not in used:
            streams.append((dl, True))
            used.update((dl, hi))
        else:
            streams.append((dl, False))
            used.add(dl)
    return streams


def _make_main_streams(deltas):
    """Main-stream specs minimizing PE passes: y-pairs via the doubled
    f2 stack, then x-pairs among the leftovers via the column-shifted g
    stack, then true singles.  spec = (kind, base_delta) with kind
    'fy' (K=128, limbs (dy,dx)+(dy+1,dx) from f3),
    'gx' (K=128, limbs (dy,dx)+(dy,dx+1) from g3),
    'f1' (K=64 single from f3).  'gx' specs go last so the g copies
    have time to land."""
    dset, used = set(deltas), set()
    fy, f1 = [], []
    for dl in sorted(deltas):
        if dl in used:
            continue
        hi = (dl[0] + 1, dl[1])
        if hi in dset and hi not in used:
            fy.append(('fy', dl))
            used.update((dl, hi))
    for dl in sorted(deltas):
        if dl not in used:
            f1.append(('f1', dl))
            used.add(dl)
    return fy + f1


def _stream_tensors(G, streams, mw=MROWS, G2=None):
    """lhsT arrays [K, mw] per stream (K=128 paired, 64 single).

    With G2, cols 64:112 carry the second stack (right-edge corr)."""
    out = []
    for dl, paired in streams:
        def block(d):
            M = np.zeros((mw, C))
            M[0:MROWS, :] = G[d]
            if G2 is not None and d in G2:
                M[RIG0:RIG0 + MROWS, :] += G2[d]
            return M
        if paired:
            hi = (dl[0] + 1, dl[1])
            lhsT = np.zeros((128, mw), np.float32)
            lhsT[0:C, :] = block(dl).T
            lhsT[C:2 * C, :] = block(hi).T
        else:
            lhsT = np.ascontiguousarray(block(dl).T, dtype=np.float32)
        out.append(lhsT)
    return out


def _plan_and_host_data(d):
    """Everything the host precomputes: stream plans, per-core inputs,
    query indices."""
    cls = _class_constants(d)
    tail_w = np.asarray(d['tail_w'], np.float64)
    tail_b = np.asarray(d['tail_b'], np.float64)

    E_main = _build_E(tail_w, cls)
    deltas = sorted({k for acc in E_main.values() for k in acc})
    dys = [dl[0] for dl in deltas]
    dxs = [dl[1] for dl in deltas]
    dy_min, dy_max = min(dys), max(dys)
    dx_min, dx_max = min(dxs), max(dxs)
    NRF = 17 + dy_max - dy_min       # f rows per core (last row upper-only)
    NCF = W + dx_max - dx_min        # f cols
    NF = NRF * NCF
    assert NRF <= 40 and NCF <= 192, (NRF, NCF)

    main_streams = _make_main_streams(deltas)
    G_main = _stack_E(E_main, deltas)
    # right-edge correction folded into main lhsT cols 64:112 (sign -1),
    # read from the matmul output at column W-1 of each row
    rig_cls = [(b, 3) for b in range(S)]
    E_rig = _build_E(tail_w, cls, only_tx=(2,))
    G_rig = _stack_E(E_rig, deltas, classes=rig_cls, sign=-1.0)

    def main_block(d):
        M = np.zeros((MW, C))
        M[0:MROWS, :] = G_main[d]
        if d in G_rig:
            M[RIG0:RIG0 + MROWS, :] += G_rig[d]
        return M

    main_T = []
    for kind, dl in main_streams:
        if kind == 'f1':
            lhsT = np.ascontiguousarray(main_block(dl).T, dtype=np.float32)
        else:
            hi = ((dl[0] + 1, dl[1]) if kind == 'fy'
                  else (dl[0], dl[1] + 1))
            lhsT = np.zeros((128, MW), np.float32)
            lhsT[0:C, :] = main_block(dl).T
            lhsT[C:2 * C, :] = main_block(hi).T
        main_T.append(lhsT)

    # remaining edge corrections as separate streams
    def corr(only_ty, only_tx, classes, sign):
        E = _build_E(tail_w, cls, only_ty=only_ty, only_tx=only_tx)
        dls = sorted({k for (ba, acc) in E.items() if ba in classes
                      for k, M in acc.items()})
        if not dls:
            return [], []
        st = _pair_streams(dls)
        G = _stack_E(E, dls, classes=classes, sign=sign)
        return st, _stream_tensors(G, st)

    top_cls = [(0, a) for a in range(S)]
    bot_cls = [(3, a) for a in range(S)]
    lef_cls = [(b, 0) for b in range(S)]
    c_lef = corr(None, (0,), lef_cls, -1.0)

    # top/bottom edge correction M-stacked with the corner add-backs:
    # lhsT cols 0:48 = edge correction; cols 64:96 / 96:128 hold the two
    # corner add-backs at 32-aligned windows (row j of window <-> pred row
    # j + base), so corner merges are 32-partition-aligned DVE adds.
    def corr_merged(only_ty, edge_cls, cornerL, cornerR, cbase):
        E_edge = _build_E(tail_w, cls, only_ty=only_ty)
        E_cl = _build_E(tail_w, cls, only_ty=only_ty, only_tx=(0,))
        E_cr = _build_E(tail_w, cls, only_ty=only_ty, only_tx=(2,))
        dls = sorted({k for ba in edge_cls for k in E_edge[ba]})
        st = _pair_streams(dls)
        G = {dl: np.zeros((128, C)) for dl in dls}
        for ba in edge_cls:
            m0 = (4 * ba[0] + ba[1]) * 3
            for dl, M in E_edge[ba].items():
                G[dl][m0:m0 + 3, :] -= M
        mL = (4 * cornerL[0] + cornerL[1]) * 3 - cbase
        for dl, M in E_cl[cornerL].items():
            G[dl][64 + mL:64 + mL + 3, :] += M
        mR = (4 * cornerR[0] + cornerR[1]) * 3 - cbase
        for dl, M in E_cr[cornerR].items():
            G[dl][96 + mR:96 + mR + 3, :] += M
        out = []
        for dl, paired in st:
            if paired:
                hi = (dl[0] + 1, dl[1])
                lhsT = np.zeros((128, 128), np.float32)
                lhsT[0:C, :] = G[dl].T
                lhsT[C:2 * C, :] = G.get(hi, np.zeros((128, C))).T
            else:
                lhsT = np.ascontiguousarray(G[dl].T, dtype=np.float32)
            out.append(lhsT)
        return st, out

    # top window base 0 (corners in pred rows 0-31), bottom base 32
    c_top = corr_merged((0,), top_cls, (0, 0), (0, 3), 0)
    c_bot = corr_merged((2,), bot_cls, (3, 0), (3, 3), 32)

    zeros_like_T = lambda Ts: [np.zeros_like(t) for t in Ts]

    # encoder weights, block-diagonal over the two row-shifted halves:
    # K = 2*28 rows = (9 taps x 3 ch + bias row) x 2, M = 128 = f | f(y+1)
    enc_w = np.asarray(d['enc_w'], np.float64)
    enc_b = np.asarray(d['enc_b'], np.float64)
    encw = np.zeros((28, C), np.float32)
    for ty in range(3):
        for tx in range(3):
            for ch in range(3):
                encw[(ty * 3 + tx) * 3 + ch, :] = enc_w[:, ch, ty, tx]
    encw[27, :] = enc_b
    encw56 = np.zeros((56, 128), np.float32)
    encw56[0:28, 0:C] = encw
    encw56[28:56, C:128] = encw

    # per-core doubled im2col [56, NF]: rows 28:56 = one LR row down
    inp = np.asarray(d['inp'], np.float64)[0]   # [3, H, W]
    PADX = 64
    ippad = np.pad(inp, ((0, 0), (PADX, PADX), (PADX, PADX)))
    ones = np.zeros((H + 2 * PADX, W + 2 * PADX))
    ones[PADX:PADX + H, PADX:PADX + W] = 1.0
    im2cols = []
    for core in range(NCORES):
        y0 = YLC * core + dy_min          # global LR row of f-tile row 0
        x0 = dx_min
        NR1 = NRF + 1
        im = np.zeros((28, NR1, NCF), np.float32)
        for ty in range(3):
            for tx in range(3):
                ys = PADX + y0 + ty - 1
                xs = PADX + x0 + tx - 1
                for ch in range(3):
                    im[(ty * 3 + tx) * 3 + ch] = \
                        ippad[ch, ys:ys + NR1, xs:xs + NCF]
        inside = ones[PADX + y0:PADX + y0 + NR1, PADX + x0:PADX + x0 + NCF]
        im[27] = inside
        # f must be exactly zero at out-of-image positions (grid-sample
        # zero padding), so kill whole columns there, not just oob taps
        im *= inside[None].astype(np.float32)
        imf = im.reshape(28, NR1 * NCF)
        im56 = np.zeros((56, NF), np.float32)
        im56[0:28] = imf[:, 0:NF]
        im56[28:56] = imf[:, NCF:NF + NCF]
        im2cols.append(im56.astype(BF16))

    # query indices (f32 math matches reference rounding); the value
    # lookup happens on host after the kernel ships pred
    coord = np.asarray(d['coord'], np.float32)[0]
    cell = np.asarray(d['cell'], np.float32)[0]
    cq = np.clip(coord - cell * np.float32(0.5) + np.float32(1e-6),
                 np.float32(-1 + 1e-6), np.float32(1 - 1e-6))
    xi = np.clip(np.round((cq[:, 1] + 1) * np.float32(0.5) * (WH - 1)
                          ).astype(np.int64), 0, WH - 1)
    yi = np.clip(np.round((cq[:, 0] + 1) * np.float32(0.5) * (HH - 1)
                          ).astype(np.int64), 0, HH - 1)

    bias48 = np.zeros((1, MROWS), np.float32)
    for b in range(S):
        for a in range(S):
            bias48[0, (4 * b + a) * 3:(4 * b + a) * 3 + 3] = tail_b

    plan = dict(
        dy_min=dy_min, dx_min=dx_min, NRF=NRF, NCF=NCF, NF=NF,
        main_streams=main_streams,
        corr_specs=dict(top=c_top[0], bot=c_bot[0], lef=c_lef[0]),
    )

    per_core_corr = []
    for core in range(NCORES):
        cc = dict(lef=c_lef[1])
        cc['top'] = c_top[1] if core == 0 else zeros_like_T(c_top[1])
        cc['bot'] = c_bot[1] if core == NCORES - 1 else zeros_like_T(c_bot[1])
        per_core_corr.append(cc)

    # ---- pack every small constant into one [128, CW] blob ----
    # segment name -> (p0, c0, K, Mw)
    segs = {}
    state = dict(col=0)

    def alloc(name, K, Mw):
        c0 = state['col']
        segs[name] = (0, c0, K, Mw)
        state['col'] += Mw
        return segs[name]

    alloc('encw', 56, 128)
    alloc('bias48', MROWS, 1)
    CW1 = state['col']               # chunk 1: encw + bias
    for s, t in enumerate(main_T):
        alloc(f'Em{s}', t.shape[0], MW)
    CW2 = state['col']               # chunk 2: Em streams
    for name, streams, mw in [('top', c_top[0], 128), ('bot', c_bot[0], 128),
                              ('lef', c_lef[0], MROWS)]:
        for s, (dl, paired) in enumerate(streams):
            alloc(f'Ec_{name}{s}', 128 if paired else C, mw)
    CW = state['col']
    plan['segs'] = segs
    plan['CW'] = CW
    plan['CW1'] = CW1
    plan['CW2'] = CW2

    consts_cores = []
    for core in range(NCORES):
        blob = np.zeros((128, CW), np.float32)

        def put(name, arr):
            p0, c0, K, Mw = segs[name]
            blob[p0:p0 + arr.shape[0], c0:c0 + arr.shape[1]] = arr

        put('encw', encw56)
        put('bias48', bias48.reshape(MROWS, 1))
        for s, t in enumerate(main_T):
            put(f'Em{s}', t)
        for name, Ts in per_core_corr[core].items():
            for s, t in enumerate(Ts):
                put(f'Ec_{name}{s}', t)
        consts_cores.append(blob.astype(BF16))

    host = dict(consts=consts_cores, im2cols=im2cols, xi=xi, yi=yi,
                Q=coord.shape[0])
    return plan, host


def _build_graph(plan, host, debug_outputs=False):
    _ensure_path()
    import concourse.bass as bass
    import concourse.bacc as bacc
    import concourse.mybir as mybir
    import concourse.tile as tile

    f32 = mybir.dt.float32
    bf16 = mybir.dt.bfloat16

    NRF, NCF, NF = plan['NRF'], plan['NCF'], plan['NF']
    dy_min, dx_min = plan['dy_min'], plan['dx_min']
    main_streams = plan['main_streams']
    corr_specs = plan['corr_specs']
    segs, CW = plan['segs'], plan['CW']

    nc = bacc.Bacc(None, target_bir_lowering=False, debug=False,
                   num_devices=NCORES)

    im2col_d = nc.dram_tensor('im2col', [56, NF], bf16, kind='ExternalInput')
    consts_d = nc.dram_tensor('consts', [128, CW], bf16,
                              kind='ExternalInput')
    out_d = nc.dram_tensor('out', [MROWS, YLC * W], bf16,
                           kind='ExternalOutput')
    if debug_outputs:
        dbg_f2 = nc.dram_tensor('dbg_f2', [128, NF], f32, kind='ExternalOutput')

    with tile.TileContext(nc) as tc:
        with (
            tc.tile_pool(name='sb', bufs=1) as sb,
            tc.tile_pool(name='sbsmall', bufs=1) as sbs,
            tc.tile_pool(name='pshare', bufs=4, space='PSUM') as pshare,
            tc.tile_pool(name='ppred', bufs=3, space='PSUM') as ppred,
            tc.tile_pool(name='pcorr', bufs=1, space='PSUM') as pcorr,
        ):
            consts_t = sb.tile([128, CW], bf16)
            im2col = sb.tile([56, NF], bf16)

            # input DMAs fan out across the three DMA-capable engines
            # (sync/scalar/gpsimd) in criticality order: the HWDGE
            # descriptor-gen (~0.8us each) serializes per engine and
            # receipts land ~2.5-3us after issue.
            CW1, CW2 = plan['CW1'], plan['CW2']
            CH = 512
            nchunks = (NF + CH - 1) // CH
            nc.sync.dma_start(im2col[:, 0:CH], im2col_d[:, 0:CH])
            nc.scalar.dma_start(consts_t[:, 0:CW1], consts_d[:, 0:CW1])
            nc.gpsimd.dma_start(consts_t[:, CW1:CW2], consts_d[:, CW1:CW2])
            nc.sync.dma_start(im2col[:, CH:2 * CH], im2col_d[:, CH:2 * CH])
            nc.scalar.dma_start(im2col[:, 2 * CH:3 * CH],
                                im2col_d[:, 2 * CH:3 * CH])
            nc.gpsimd.dma_start(im2col[:, 3 * CH:4 * CH],
                                im2col_d[:, 3 * CH:4 * CH])
            nc.sync.dma_start(im2col[:, 4 * CH:NF], im2col_d[:, 4 * CH:NF])
            nc.scalar.dma_start(consts_t[:, CW2:CW], consts_d[:, CW2:CW])

            # warm-up matmuls fill the input-DMA wait: the PE clock ramps
            # with activity (~0.8 GHz cold -> ~1.2 GHz warm), so burning
            # the dead time on dummy matmuls pulls the fast clock earlier
            warm = sbs.tile([128, 512], bf16)
            warmw = sbs.tile([128, 128], bf16)
            nc.vector.memset(warm[:], 0)
            nc.vector.memset(warmw[:], 0)
            for _ in range(8):
                pw = pshare.tile([128, 512], f32, tag='pshare')
                nc.tensor.matmul(pw[:], warmw[:], warm[:],
                                 start=True, stop=True,
                                 skip_group_check=True)

            def cseg(name):
                p0, c0, K, Mw = segs[name]
                return consts_t[p0:p0 + K, c0:c0 + Mw]

            encw_t = cseg('encw')
            bias48_t = cseg('bias48')
            biasf_t = sbs.tile([MROWS, 1], f32)
            mainT_t = [cseg(f'Em{s}') for s in range(len(main_streams))]
            corrT_t = {name: [cseg(f'Ec_{name}{s}')
                              for s in range(len(streams))]
                       for name, streams in corr_specs.items()}

            # encoder conv: f2 = [f ; f shifted one LR row] in one
            # block-diagonal K=56 matmul per chunk.  The PSUM->SBUF cast
            # splits across vector and scalar: big SBUF writes slow
            # concurrent main-matmul SBUF reads ~1.5x, so compressing
            # the cast chain shortens the contention window.
            f2 = sb.tile([128, NF], bf16)

            def enc_chunk(ci):
                n0, n1 = ci * CH, min(NF, (ci + 1) * CH)
                nh = (n1 - n0) // 2
                pe = pshare.tile([128, CH], f32, tag='pshare')
                nc.tensor.matmul(pe[:, :n1 - n0],
                                 encw_t,
                                 im2col[:, n0:n1],
                                 start=True, stop=True,
                                 skip_group_check=True)
                nc.vector.tensor_copy(f2[:, n0:n0 + nh], pe[:, :nh])
                nc.scalar.activation(f2[:, n0 + nh:n1], pe[:, nh:n1 - n0],
                                     mybir.ActivationFunctionType.Copy)

            f3 = f2[:].rearrange('p (r c) -> p r c', c=NCF)

            def main_mms(nb, pred_ps):
                for s, (kind, dl) in enumerate(main_streams):
                    K = C if kind == 'f1' else 128
                    r0 = 4 * nb + dl[0] - dy_min
                    c0 = dl[1] - dx_min
                    nc.tensor.matmul(
                        pred_ps[:],
                        mainT_t[s],
                        f3[0:K, r0:r0 + 4, c0:c0 + W],
                        start=(s == 0), stop=(s == len(main_streams) - 1),
                        skip_group_check=True)

            # all encoder chunks (and their casts) run before the mains:
            # concurrent PSUM-reading casts slow the main matmul streams
            # ~1.5x, so the cast chain must drain first
            for ci in range(nchunks):
                enc_chunk(ci)
            pred_ps0 = ppred.tile([MW, 512], f32, tag='ppred')
            main_mms(0, pred_ps0)

            # corrections first: they only need f2, and every bank's
            # merge depends on them; top/bot (with corner windows) and
            # lef share one PSUM bank
            corrall = pcorr.tile([128, 512], f32)
            corr2_ps = corrall[:, 0:256]
            corr_ps = corrall[0:MROWS, 256:272]

            def corr_mms(name, col0, row_sel, col_sel, nfree, ps):
                streams = corr_specs[name]
                if not streams:
                    return False
                for s, (dl, paired) in enumerate(streams):
                    K = 128 if paired else C
                    r0 = row_sel + dl[0] - dy_min
                    c0 = col_sel + dl[1] - dx_min
                    if nfree == 128:     # one row, all cols
                        rhs = f3[0:K, r0:r0 + 1, c0:c0 + W]
                    else:                # all rows, one col
                        rhs = f3[0:K, r0:r0 + YLC, c0:c0 + 1]
                    nc.tensor.matmul(
                        ps[:, col0:col0 + nfree],
                        corrT_t[name][s],
                        rhs,
                        start=(s == 0), stop=(s == len(streams) - 1),
                        skip_group_check=True)
                return True

            has = dict()
            has['top'] = corr_mms('top', 0, 0, 0, 128, corr2_ps)
            has['bot'] = corr_mms('bot', 128, YLC - 1, 0, 128, corr2_ps)
            has['lef'] = corr_mms('lef', 0, 0, 0, 16, corr_ps)

            # emitted here (not at consts load) so the wait on the consts
            # receipt never head-of-line-blocks the encoder copies
            nc.vector.tensor_copy(biasf_t[:], bias48_t)

            # fused per-bank pipeline: main matmuls -> copy+merge -> out
            # store; pred PSUM is double-buffered so the next bank's
            # matmuls overlap this bank's merge reads.  Banks run in
            # order [0,1,3,2] so the bottom-edge bank (3, with its extra
            # corner adds) is never the last one on the critical tail.
            pred_sb = sb.tile([MROWS, YLC * W], bf16)
            p4 = pred_sb[:].rearrange('p (r c) -> p r c', c=W)

            for nb in (0, 1, 3, 2):
                if nb == 0:
                    pred_ps = pred_ps0
                else:
                    pred_ps = ppred.tile([MW, 512], f32, tag='ppred')
                    main_mms(nb, pred_ps)
                rig4 = pred_ps[RIG0:RIG0 + MROWS, :].rearrange(
                    'p (r c) -> p r c', c=W)
                # both half-merges issue first (scalar || vector), then
                # the edge adds, so the add chain never waits on a merge
                r0a, r0b = 4 * nb, 4 * nb + 2
                nc.scalar.activation(
                    pred_sb[:, r0a * W:(r0a + 2) * W],
                    pred_ps[0:MROWS, 0:256],
                    mybir.ActivationFunctionType.Identity,
                    bias=biasf_t[:])
                nc.vector.tensor_scalar_add(
                    pred_sb[:, r0b * W:(r0b + 2) * W],
                    pred_ps[0:MROWS, 256:512],
                    biasf_t[:])
                for half in range(2):
                    r0, r1 = 4 * nb + 2 * half, 4 * nb + 2 * half + 2
                    if has['lef']:
                        nc.vector.tensor_add(
                            p4[:, r0:r1, 0:1],
                            p4[:, r0:r1, 0:1],
                            corr_ps[:, r0:r1]
                            .rearrange('p (r c) -> p r c', c=1))
                    # right-edge correction rides the main matmuls (M 64:112)
                    nc.vector.tensor_add(
                        p4[:, r0:r1, W - 1:W],
                        p4[:, r0:r1, W - 1:W],
                        rig4[:, 2 * half:2 * half + 2, W - 1:W])
                    if nb == 0 and half == 0 and has['top']:
                        nc.vector.tensor_add(pred_sb[:, 0:W],
                                             pred_sb[:, 0:W],
                                             corr2_ps[0:MROWS, 0:W])
                        nc.vector.tensor_add(
                            p4[0:32, 0:1, 0:1], p4[0:32, 0:1, 0:1],
                            corr2_ps[64:96, 0:1]
                            .rearrange('p (r c) -> p r c', c=1))
                        nc.vector.tensor_add(
                            p4[0:32, 0:1, W - 1:W], p4[0:32, 0:1, W - 1:W],
                            corr2_ps[96:128, W - 1:W]
                            .rearrange('p (r c) -> p r c', c=1))
                    if nb == 3 and half == 1 and has['bot']:
                        nc.vector.tensor_add(
                            pred_sb[:, (YLC - 1) * W:YLC * W],
                            pred_sb[:, (YLC - 1) * W:YLC * W],
                            corr2_ps[0:MROWS, 128:128 + W])
                        nc.vector.tensor_add(
                            p4[32:48, YLC - 1:YLC, 0:1],
                            p4[32:48, YLC - 1:YLC, 0:1],
                            corr2_ps[64:80, 128:129]
                            .rearrange('p (r c) -> p r c', c=1))
                        nc.vector.tensor_add(
                            p4[32:48, YLC - 1:YLC, W - 1:W],
                            p4[32:48, YLC - 1:YLC, W - 1:W],
                            corr2_ps[96:112, 255:256]
                            .rearrange('p (r c) -> p r c', c=1))
                    # out stores alternate queues so the last bank's two
                    # halves don't serialize on one descriptor generator
                    eng = nc.sync if half == 0 else nc.scalar
                    eng.dma_start(
                        out_d[:, r0 * W:r1 * W],
                        pred_sb[:, r0 * W:r1 * W])

            if debug_outputs:
                nc.sync.dma_start(dbg_f2[:], f2[:].bitcast(f32))

    nc.compile()
    return nc


def make_in_maps(host):
    in_maps = []
    for core in range(NCORES):
        m = {
            'im2col': host['im2cols'][core],
            'consts': host['consts'][core],
        }
        in_maps.append(m)
    return in_maps


def kernel(**inputs) -> np.ndarray:
    _ensure_path()
    from concourse.bass_utils import run_bass_kernel_spmd

    scale = inputs.get('scale', S)
    scale = int(np.asarray(scale)) if not isinstance(scale, int) else scale
    assert scale == S, f"kernel hardcodes scale={S}, got {scale}"

    plan, host = _plan_and_host_data(inputs)
    nc = _build_graph(plan, host)

    in_maps = make_in_maps(host)
    res = run_bass_kernel_spmd(nc, in_maps, core_ids=list(range(NCORES)))

    # assemble pred [3, HH, WH] from the per-core [48, YLC*W] tiles:
    # row (4b+a)*3+c, col yl*W+xl  ->  pred[c, HRPC*core + 4*yl + b, 4*xl + a]
    pred = np.empty((3, HH, WH), np.float32)
    for core in range(NCORES):
        t = np.asarray(res.results[core]['out']).astype(np.float32)
        t = t.reshape(S, S, 3, YLC, W)            # [b, a, c, yl, xl]
        pred[:, HRPC * core:HRPC * (core + 1), :] = (
            t.transpose(2, 3, 0, 4, 1).reshape(3, HRPC, WH))
    q = pred[:, host['yi'], host['xi']].T         # [Q, 3]
    return q[None]
